# revision 1
# baseline (speedup 1.0000x reference)
"""Trainium2 Bass kernel for sparse multi-head edge attention.

Computation (per the nn.Module):
    Q = Fa @ Wq.T, K = Fb @ Wk.T, V = Fb @ Wv.T   (reshaped to H=8 heads x 32)
    per edge e: logit[e,h] = <Q[a_e,h,:], K[b_e,h,:]> / sqrt(32)
    segmented softmax over edges per query, out = Fa + (softmax-weighted V) @ Wproj.T

Strategy (8 NeuronCores, SPMD, no collectives):
  - Shard queries: core m owns rows [m*6250, (m+1)*6250). Every core gets the
    full Fb (K/V tables are built redundantly); the segmented softmax is fully
    core-local.
  - Max |logit| is ~10 for this operator family (inner products of unit-normal
    features), so exp() is computed WITHOUT the max-subtraction: softmax is
    shift invariant and fp32 exp is safe up to ~88.  Both segment reductions
    (sum of exp, sum of exp*V) are then plain segmented SUMS, computed on the
    TensorEngine as one-hot selection matmuls accumulated in PSUM (one fused
    [den|num] matmul per 128-edge tile).
  - K|V are built as ONE fused fp16 table row (1KB) so each edge needs a
    single dma_gather descriptor.  The GPSIMD Q7 descriptor-generation cost
    (~8ns/row) is the kernel's critical resource, so Q rows are NOT gathered:
    Q stays resident in SBUF and per-edge Q rows are materialized on the
    TensorEngine as Qe = selT.T @ Qblk, where selT is built by comparing a
    host-streamed row-replicated a_rel array against the partition index.
  - dma_gather's int16 row index limit (<=32767) is handled by splitting the
    KV table at row 32768; each block's edges are sorted into a "lo" stream
    and a "hi" stream, each padded to a multiple of 128 slots so the two
    gathers write disjoint column ranges of the same SBUF tile.
  - Pad edges point at row 0 with an exp-bias of -1e5 so they contribute
    exactly 0; queries with no edges produce 0 attention output (den is
    clamped with max(den, 1e-30) like the reference).
"""

import math

import numpy as np

P = 128
H = 8
DH = 32
CDIM = 256  # feature/channel dim (CA = CB = D = 256)
NA = 50000
NB = 50000
NCORES = 8
NAC = NA // NCORES          # 6250 queries per core
NBLK = (NAC + P - 1) // P   # 49 query blocks per core
NPADQ = NBLK * P            # 6272 padded queries per core
SPLIT = 32768               # int16-safe table split
KV_ROWS = ((NB + P - 1) // P) * P   # 50048
KVHI_ROWS = KV_ROWS - SPLIT         # 17280
CHUNK = 2048                # rows per table-build chunk (16 sub-blocks)
SCALE = 1.0 / math.sqrt(DH)
PAD_BIAS = -1.0e5

F16 = np.float16
F32 = np.float32


def _ceil128(x):
    return (np.asarray(x) + P - 1) // P * P


def preprocess(Fa, Fb, a_idx, b_idx, Wq, Wk, Wv, Wproj):
    """Host-side sharding: returns (meta, shared_inputs, per_core_inputs)."""
    a_idx = np.asarray(a_idx).astype(np.int64)
    b_idx = np.asarray(b_idx).astype(np.int64)
    Fa = np.asarray(Fa, F32)
    Fb = np.asarray(Fb, F32)

    core = a_idx // NAC
    a_loc = a_idx - core * NAC
    blk = a_loc // P
    a_rel_v = a_loc % P
    hi = b_idx >= SPLIT

    # per (core, block) lo/hi counts -> shared static capacities
    cnt_lo = np.zeros((NCORES, NBLK), np.int64)
    cnt_hi = np.zeros((NCORES, NBLK), np.int64)
    np.add.at(cnt_lo, (core[~hi], blk[~hi]), 1)
    np.add.at(cnt_hi, (core[hi], blk[hi]), 1)
    LO = _ceil128(cnt_lo.max(axis=0))
    HI = _ceil128(cnt_hi.max(axis=0))
    CAP = LO + HI
    coff = np.concatenate([[0], np.cumsum(CAP)])        # edge-slot offsets
    loff = np.concatenate([[0], np.cumsum(LO)])
    hoff = np.concatenate([[0], np.cumsum(HI)])
    TOT = int(coff[-1])          # edge slots per core
    TC = TOT // P                # tile columns per core
    TOTLO = int(loff[-1])
    TOTHI = int(hoff[-1])

    # rank of each edge within its (core, blk, half) group
    ne = a_idx.shape[0]
    gid = (core * NBLK + blk) * 2 + hi.astype(np.int64)
    order = np.argsort(gid, kind="stable")
    counts = np.bincount(gid, minlength=NCORES * NBLK * 2)
    gstart = np.concatenate([[0], np.cumsum(counts)])[:-1]
    rank = np.empty(ne, np.int64)
    rank[order] = np.arange(ne) - gstart[gid[order]]

    # slot within the core's edge stream
    slot = np.where(hi, coff[blk] + LO[blk] + rank, coff[blk] + rank)
    kv_slot = np.where(hi, hoff[blk] + rank, loff[blk] + rank)

    kvlo_idx = np.zeros((NCORES, TOTLO), np.int16)
    kvhi_idx = np.zeros((NCORES, TOTHI), np.int16)
    a_rel = np.zeros((NCORES, TOT), F16)
    bias = np.full((NCORES, TOT), PAD_BIAS, F32)

    a_rel[core, slot] = a_rel_v.astype(F16)
    bias[core, slot] = 0.0
    lo_m = ~hi
    kvlo_idx[core[lo_m], kv_slot[lo_m]] = b_idx[lo_m].astype(np.int16)
    kvhi_idx[core[hi], kv_slot[hi]] = (b_idx[hi] - SPLIT).astype(np.int16)

    def wrap16(arr):  # [N] -> [128, N/16] (16-slot wrap replicated 8x)
        w = arr.reshape(-1, 16).T
        return np.tile(w, (8, 1)).copy()

    def slots128(arr):  # [TOT] -> [128, TC]; slot i -> (i%128, i//128)
        return arr.reshape(-1, P).T.copy()

    FbT = np.zeros((CDIM, KV_ROWS), F16)
    FbT[:, :NB] = Fb.T.astype(F16)

    shared = {
        "FbT": FbT,
        "WqT": Wq.T.astype(F16).copy(),
        # fused [K|V] projection: rhs for the N=512 table-build matmuls
        "WKVT": np.concatenate([Wk.T, Wv.T], axis=1).astype(F16).copy(),
        "WprojT": Wproj.T.astype(F16).copy(),
        "IOTA": np.tile(np.arange(P, dtype=F16), (P, 1)).copy(),
        "IOTACOL": np.arange(P, dtype=F16).reshape(P, 1).copy(),
        "IDENT": np.eye(P, dtype=F16),
    }

    per_core = []
    for m in range(NCORES):
        FaT = np.zeros((CDIM, NPADQ), F16)
        FaT[:, :NAC] = Fa[m * NAC:(m + 1) * NAC].T.astype(F16)
        Fa_res = np.zeros((NPADQ, CDIM), F32)
        Fa_res[:NAC] = Fa[m * NAC:(m + 1) * NAC]
        arel_m = a_rel[m]
        per_core.append({
            "FaT": FaT,
            "FaRes": Fa_res,
            "KVLOIDX": wrap16(kvlo_idx[m]) if TOTLO else np.zeros((P, 0), np.int16),
            "KVHIIDX": wrap16(kvhi_idx[m]) if TOTHI else np.zeros((P, 0), np.int16),
            "AREL": slots128(arel_m),
            # row-replicated a_rel in slot order, streamed per block for selT
            "ARELT": np.broadcast_to(arel_m[None, :], (P, TOT)).copy(),
            "BIAS": slots128(bias[m]),
        })

    meta = {
        "LO": LO.astype(int), "HI": HI.astype(int), "CAP": CAP.astype(int),
        "coff": coff.astype(int), "loff": loff.astype(int),
        "hoff": hoff.astype(int), "TOT": TOT, "TC": TC,
        "TOTLO": TOTLO, "TOTHI": TOTHI,
    }
    return meta, shared, per_core


def build_program(meta):
    import concourse.bacc as bacc
    import concourse.mybir as mybir
    from concourse.tile import TileContext
    from concourse import library_config

    dt = mybir.dt
    nc = bacc.Bacc("TRN2", target_bir_lowering=False, debug=False,
                   num_devices=NCORES)

    TC = meta["TC"]
    TOT = meta["TOT"]
    TOTLO, TOTHI = meta["TOTLO"], meta["TOTHI"]
    LO, CAP, coff = meta["LO"], meta["CAP"], meta["coff"]
    loff, hoff = meta["loff"], meta["hoff"]

    # ---- I/O ----
    FbT_t = nc.dram_tensor("FbT", [CDIM, KV_ROWS], dt.float16, kind="ExternalInput")
    FaT_t = nc.dram_tensor("FaT", [CDIM, NPADQ], dt.float16, kind="ExternalInput")
    FaRes_t = nc.dram_tensor("FaRes", [NPADQ, CDIM], dt.float32, kind="ExternalInput")
    WqT_t = nc.dram_tensor("WqT", [CDIM, CDIM], dt.float16, kind="ExternalInput")
    WKVT_t = nc.dram_tensor("WKVT", [CDIM, 2 * CDIM], dt.float16, kind="ExternalInput")
    WprojT_t = nc.dram_tensor("WprojT", [CDIM, CDIM], dt.float16, kind="ExternalInput")
    IOTA_t = nc.dram_tensor("IOTA", [P, P], dt.float16, kind="ExternalInput")
    IOTACOL_t = nc.dram_tensor("IOTACOL", [P, 1], dt.float16, kind="ExternalInput")
    IDENT_t = nc.dram_tensor("IDENT", [P, P], dt.float16, kind="ExternalInput")
    KVLO_I_t = nc.dram_tensor("KVLOIDX", [P, max(TOTLO // 16, 1)], dt.int16,
                              kind="ExternalInput")
    KVHI_I_t = nc.dram_tensor("KVHIIDX", [P, max(TOTHI // 16, 1)], dt.int16,
                              kind="ExternalInput")
    AREL_t = nc.dram_tensor("AREL", [P, TC], dt.float16, kind="ExternalInput")
    ARELT_t = nc.dram_tensor("ARELT", [P, TOT], dt.float16, kind="ExternalInput")
    BIAS_t = nc.dram_tensor("BIAS", [P, TC], dt.float32, kind="ExternalInput")

    KVlo = nc.dram_tensor("KVlo", [SPLIT, 2 * CDIM], dt.float16, kind="Internal")
    KVhi = nc.dram_tensor("KVhi", [KVHI_ROWS, 2 * CDIM], dt.float16, kind="Internal")
    OUT_t = nc.dram_tensor("OUT", [NPADQ, CDIM], dt.float32, kind="ExternalOutput")

    CMAX = int(CAP.max()) // P
    AluOp = mybir.AluOpType

    with TileContext(nc) as tc:
        # dma_gather lives in the "mlp" GPSIMD ucode library; load it before
        # any Pool-engine work (first-emitted => first on the Pool engine).
        nc.gpsimd.load_library(library_config.mlp)
        with tc.tile_pool(name="res", bufs=1) as rpool:
            # resident constants / metadata
            wq = rpool.tile([P, 2, CDIM], dt.float16, tag="wq")
            wkv = rpool.tile([P, 2, 2 * CDIM], dt.float16, tag="wkv")
            wproj = rpool.tile([P, 2, CDIM], dt.float16, tag="wproj")
            nc.sync.dma_start(out=wq[:, 0, :], in_=WqT_t[0:P, :])
            nc.sync.dma_start(out=wq[:, 1, :], in_=WqT_t[P:2 * P, :])
            nc.sync.dma_start(out=wkv[:, 0, :], in_=WKVT_t[0:P, :])
            nc.sync.dma_start(out=wkv[:, 1, :], in_=WKVT_t[P:2 * P, :])
            nc.sync.dma_start(out=wproj[:, 0, :], in_=WprojT_t[0:P, :])
            nc.sync.dma_start(out=wproj[:, 1, :], in_=WprojT_t[P:2 * P, :])
            iota = rpool.tile([P, P], dt.float16, tag="iota")
            iotacol = rpool.tile([P, 1], dt.float16, tag="iotacol")
            ident = rpool.tile([P, P], dt.float16, tag="ident")
            nc.sync.dma_start(out=iota[:], in_=IOTA_t[:, :])
            nc.sync.dma_start(out=iotacol[:], in_=IOTACOL_t[:, :])
            nc.sync.dma_start(out=ident[:], in_=IDENT_t[:, :])
            kvloidx = rpool.tile([P, max(TOTLO // 16, 1)], dt.int16, tag="kvloidx")
            nc.sync.dma_start(out=kvloidx[:], in_=KVLO_I_t[:, :])
            kvhiidx = rpool.tile([P, max(TOTHI // 16, 1)], dt.int16, tag="kvhiidx")
            nc.sync.dma_start(out=kvhiidx[:], in_=KVHI_I_t[:, :])
            arel = rpool.tile([P, TC], dt.float16, tag="arel")
            nc.sync.dma_start(out=arel[:], in_=AREL_t[:, :])
            bias = rpool.tile([P, TC], dt.float32, tag="bias")
            nc.sync.dma_start(out=bias[:], in_=BIAS_t[:, :])
            # Q table: SBUF-resident, never leaves the chip
            qres = rpool.tile([P, NBLK, CDIM], dt.float16, tag="qres")

            # ---- Phase A: build Q (to SBUF) and the fused KV table (DRAM) ----
            with tc.tile_pool(name="bld", bufs=2) as bpool, \
                 tc.tile_pool(name="psA", bufs=4, space="PSUM") as psA:
                # Q: 49 row-blocks
                for c0 in range(0, NPADQ, CHUNK):
                    nsub = min(CHUNK, NPADQ - c0) // P
                    ft = bpool.tile([P, 2, CHUNK], dt.float16, tag="ft")
                    nc.sync.dma_start(out=ft[:, 0, :nsub * P], in_=FaT_t[0:P, c0:c0 + nsub * P])
                    nc.sync.dma_start(out=ft[:, 1, :nsub * P], in_=FaT_t[P:2 * P, c0:c0 + nsub * P])
                    for s in range(nsub):
                        ps = psA.tile([P, 2 * CDIM], dt.float32, tag="psA")
                        nc.tensor.matmul(ps[:, 0:CDIM], ft[:, 0, s * P:(s + 1) * P],
                                         wq[:, 0, :], start=True, stop=False)
                        nc.tensor.matmul(ps[:, 0:CDIM], ft[:, 1, s * P:(s + 1) * P],
                                         wq[:, 1, :], start=False, stop=True)
                        nc.scalar.copy(out=qres[:, c0 // P + s, :], in_=ps[:, 0:CDIM])
                # KV fused rows; lo chunks first so lo gathers can start early
                for c0 in range(0, KV_ROWS, CHUNK):
                    nsub = min(CHUNK, KV_ROWS - c0) // P
                    ft = bpool.tile([P, 2, CHUNK], dt.float16, tag="ft")
                    nc.sync.dma_start(out=ft[:, 0, :nsub * P], in_=FbT_t[0:P, c0:c0 + nsub * P])
                    nc.sync.dma_start(out=ft[:, 1, :nsub * P], in_=FbT_t[P:2 * P, c0:c0 + nsub * P])
                    ob = bpool.tile([P, CHUNK // P, 2 * CDIM], dt.float16, tag="ob")
                    for s in range(nsub):
                        ps = psA.tile([P, 2 * CDIM], dt.float32, tag="psA")
                        nc.tensor.matmul(ps[:], ft[:, 0, s * P:(s + 1) * P],
                                         wkv[:, 0, :], start=True, stop=False)
                        nc.tensor.matmul(ps[:], ft[:, 1, s * P:(s + 1) * P],
                                         wkv[:, 1, :], start=False, stop=True)
                        nc.scalar.copy(out=ob[:, s, :], in_=ps[:])
                    if c0 < SPLIT:
                        dst_ap = KVlo[c0:c0 + nsub * P, :]
                    else:
                        dst_ap = KVhi[c0 - SPLIT:c0 - SPLIT + nsub * P, :]
                    nc.sync.dma_start(out=dst_ap.rearrange("(s p) d -> p s d", p=P),
                                      in_=ob[:, :nsub, :])

            # ---- Phase B: edge attention per query block ----
            with tc.tile_pool(name="gat", bufs=3) as gpool, \
                 tc.tile_pool(name="wrk", bufs=4) as wpool, \
                 tc.tile_pool(name="fin", bufs=2) as fpool, \
                 tc.tile_pool(name="psB", bufs=2, space="PSUM") as psB:
                for j in range(NBLK):
                    Cj = int(CAP[j]) // P
                    LOc = int(LO[j]) // P
                    kve = gpool.tile([P, CMAX, 2 * CDIM], dt.float16, tag="kve")
                    arelT = gpool.tile([P, CMAX * P], dt.float16, tag="arelT")
                    nc.sync.dma_start(out=arelT[:, :Cj * P],
                                      in_=ARELT_t[:, int(coff[j]):int(coff[j]) + Cj * P])
                    if LOc:
                        nc.gpsimd.dma_gather(
                            out_ap=kve[:, 0:LOc, :], in_ap=KVlo[:, :],
                            idxs_ap=kvloidx[:, int(loff[j]) // 16:(int(loff[j]) + int(LO[j])) // 16],
                            num_idxs=int(LO[j]), num_idxs_reg=int(LO[j]),
                            elem_size=2 * CDIM, single_packet=False)
                    if Cj - LOc:
                        hj = int(CAP[j] - LO[j])
                        nc.gpsimd.dma_gather(
                            out_ap=kve[:, LOc:Cj, :], in_ap=KVhi[:, :],
                            idxs_ap=kvhiidx[:, int(hoff[j]) // 16:(int(hoff[j]) + hj) // 16],
                            num_idxs=hj, num_idxs_reg=hj,
                            elem_size=2 * CDIM, single_packet=False)

                    dn_ps = psB.tile([P, H + CDIM], dt.float32, tag="dn")
                    for t in range(Cj):
                        g = int(coff[j]) // P + t
                        selT = wpool.tile([P, P], dt.float16, tag="selT")
                        nc.vector.tensor_tensor(
                            out=selT[:], in0=iotacol[:, 0:1].to_broadcast([P, P]),
                            in1=arelT[:, t * P:(t + 1) * P], op=AluOp.is_equal)
                        sel = wpool.tile([P, P], dt.float16, tag="sel")
                        nc.vector.tensor_tensor(
                            out=sel[:], in0=arel[:, g:g + 1].to_broadcast([P, P]),
                            in1=iota[:], op=AluOp.is_equal)
                        qe_ps = psB.tile([P, CDIM], dt.float32, tag="qe_ps")
                        nc.tensor.matmul(qe_ps[:], selT[:], qres[:, j, :],
                                         start=True, stop=True)
                        qe_sb = wpool.tile([P, CDIM], dt.float16, tag="qe_sb")
                        nc.scalar.copy(out=qe_sb[:], in_=qe_ps[:])
                        prod = wpool.tile([P, CDIM], dt.float16, tag="prod")
                        nc.vector.tensor_tensor(
                            out=prod[:], in0=qe_sb[:], in1=kve[:, t, 0:CDIM],
                            op=AluOp.mult)
                        logits = wpool.tile([P, H], dt.float32, tag="logits")
                        nc.vector.tensor_reduce(
                            out=logits[:], in_=prod[:].rearrange("p (h d) -> p h d", d=DH),
                            axis=mybir.AxisListType.X, op=AluOp.add)
                        exwv = wpool.tile([P, H + CDIM], dt.float16, tag="exwv")
                        nc.scalar.activation(
                            out=exwv[:, 0:H], in_=logits[:],
                            func=mybir.ActivationFunctionType.Exp,
                            bias=bias[:, g:g + 1], scale=SCALE)
                        nc.vector.tensor_tensor(
                            out=exwv[:, H:H + CDIM], in0=kve[:, t, CDIM:2 * CDIM],
                            in1=exwv[:, 0:H].unsqueeze(2).to_broadcast([P, H, DH]),
                            op=AluOp.mult)
                        nc.tensor.matmul(dn_ps[:], sel[:], exwv[:],
                                         start=(t == 0), stop=(t == Cj - 1))

                    # block finalize
                    den = fpool.tile([P, H], dt.float32, tag="den_sb")
                    nc.vector.tensor_scalar_max(out=den[:], in0=dn_ps[:, 0:H], scalar1=1e-30)
                    rec = fpool.tile([P, H], dt.float32, tag="rec")
                    nc.vector.reciprocal(out=rec[:], in_=den[:])
                    s_sb = fpool.tile([P, CDIM], dt.float16, tag="s_sb")
                    nc.vector.tensor_tensor(
                        out=s_sb[:], in0=dn_ps[:, H:H + CDIM],
                        in1=rec[:].unsqueeze(2).to_broadcast([P, H, DH]),
                        op=AluOp.mult)
                    st_ps = psB.tile([P, 2, P], dt.float16, tag="st_ps")
                    nc.tensor.transpose(st_ps[:, 0, :], s_sb[:, 0:P], ident[:])
                    nc.tensor.transpose(st_ps[:, 1, :], s_sb[:, P:2 * P], ident[:])
                    st_sb = fpool.tile([P, 2, P], dt.float16, tag="st_sb")
                    nc.scalar.copy(out=st_sb[:], in_=st_ps[:])
                    out_ps = psB.tile([P, CDIM], dt.float32, tag="out_ps")
                    nc.tensor.matmul(out_ps[:], st_sb[:, 0, :], wproj[:, 0, :],
                                     start=True, stop=False)
                    nc.tensor.matmul(out_ps[:], st_sb[:, 1, :], wproj[:, 1, :],
                                     start=False, stop=True)
                    fa_t = fpool.tile([P, CDIM], dt.float32, tag="fa_t")
                    nc.sync.dma_start(out=fa_t[:], in_=FaRes_t[j * P:(j + 1) * P, :])
                    res = fpool.tile([P, CDIM], dt.float32, tag="res")
                    nc.vector.tensor_tensor(out=res[:], in0=out_ps[:], in1=fa_t[:],
                                            op=AluOp.add)
                    nc.sync.dma_start(out=OUT_t[j * P:(j + 1) * P, :], in_=res[:])

    nc.compile()
    return nc


TRACE = False          # set by test harness for NTFF profiling
LAST_RESULT = None     # BassKernelResults of the last run (for profiling)


def kernel(**inputs):
    global LAST_RESULT
    from concourse.bass_utils import run_bass_kernel_spmd

    meta, shared, per_core = preprocess(**inputs)
    nc = build_program(meta)
    in_maps = [dict(shared, **pc) for pc in per_core]
    res = run_bass_kernel_spmd(nc, in_maps, core_ids=list(range(NCORES)),
                               trace=TRACE)
    LAST_RESULT = res
    out = np.empty((NA, CDIM), F32)
    for m in range(NCORES):
        out[m * NAC:(m + 1) * NAC] = res.results[m]["OUT"][:NAC]
    return out



# revision 10
# speedup vs baseline: 1.1493x; 1.1493x over previous
"""Trainium2 Bass kernel for sparse multi-head edge attention (V3).

Computation (per the nn.Module):
    Q = Fa @ Wq.T, K = Fb @ Wk.T, V = Fb @ Wv.T   (H=8 heads x 32)
    per edge e: logit[e,h] = <Q[a_e,h,:], K[b_e,h,:]> / sqrt(32)
    segmented softmax over edges per query, out = Fa + (softmax(V)) @ Wproj.T

Strategy (8 NeuronCores, SPMD, query-sharded, no collectives):
  - Core m owns queries [m*6250, (m+1)*6250).  Edges are routed to the owner
    of their query, grouped by 128-query block, padded to 128-edge tiles.
  - NO gather anywhere: the host distributes per-edge Fb rows (edge-sharded
    input layout, FbET = Fb[b_e].T in slot order) and the kernel recomputes
    K|V per edge with one fused matmul pair per 128-edge tile
    (KVe = FbE @ [Wk.T|Wv.T]).  This trades 2x matmul FLOPs for zero GPSIMD
    descriptor generation and no DRAM KV table round-trip — the previous
    dma_gather design was bottlenecked by ~10ns/row descriptor costs.
  - Per-edge Q rows come from a one-hot selection matmul (selT.T @ Qblk)
    with host-precomputed one-hot SEL/SELT streams (also encodes padding:
    pad slots have all-zero one-hot columns so they contribute exactly 0).
  - Work is spread across all four compute engines and batched in groups of
    G=2 tiles so per-instruction overheads amortize:
      PE:     KV matmuls, Qe selection, [den|num] accumulation, projection
      Scalar: Qe PSUM->SBUF copy, exp()
      DVE:    Q*K product, exp-weighted V
      GPSIMD: per-head logit reduction
  - Softmax without max-subtraction (|logit| <~ 10 for this operator family,
    fp32/fp16 exp is safe); den clamped like the reference.
"""

import math

import numpy as np

P = 128
H = 8
DH = 32
CDIM = 256
NA = 50000
NB = 50000
NCORES = 8
NAC = NA // NCORES          # 6250 queries per core
NBLK = (NAC + P - 1) // P   # 49 query blocks per core
NPADQ = NBLK * P            # 6272 padded queries per core
CHUNK = 2048
G = 2                       # tiles per op-batching group
SCALE = 1.0 / math.sqrt(DH)

F16 = np.float16
F32 = np.float32


def _ceil128(x):
    return (np.asarray(x) + P - 1) // P * P


def preprocess(Fa, Fb, a_idx, b_idx, Wq, Wk, Wv, Wproj):
    """Host-side sharding: returns (meta, shared_inputs, per_core_inputs)."""
    a_idx = np.asarray(a_idx).astype(np.int64)
    b_idx = np.asarray(b_idx).astype(np.int64)
    Fa = np.asarray(Fa, F32)
    Fb = np.asarray(Fb, F32)

    core = a_idx // NAC
    a_loc = a_idx - core * NAC
    blk = a_loc // P
    arel = a_loc % P

    cnt = np.bincount(core * NBLK + blk, minlength=NCORES * NBLK)
    cnt = cnt.reshape(NCORES, NBLK)
    CAP = np.maximum(_ceil128(cnt.max(axis=0)), P)
    coff = np.concatenate([[0], np.cumsum(CAP)])
    TOT = int(coff[-1])

    # rank of each edge within its (core, blk) group
    ne = a_idx.shape[0]
    gid = core * NBLK + blk
    order = np.argsort(gid, kind="stable")
    counts = np.bincount(gid, minlength=NCORES * NBLK)
    gstart = np.concatenate([[0], np.cumsum(counts)])[:-1]
    rank = np.empty(ne, np.int64)
    rank[order] = np.arange(ne) - gstart[gid[order]]
    slot = coff[blk] + rank

    Fb16 = Fb.astype(F16)
    wkvT = np.concatenate([Wk.T, Wv.T], axis=1)          # [256, 512]
    shared = {
        "WqT": Wq.T.astype(F16).copy(),
        # contraction-half layout [k=128, half=2, n=512]
        "WKVT": wkvT.reshape(2, P, 2 * CDIM).transpose(1, 0, 2).astype(F16).copy(),
        "WprojT": Wproj.T.astype(F16).copy(),
        "IDENT": np.eye(P, dtype=F32),
    }

    per_core = []
    for m in range(NCORES):
        msk = core == m
        sl = slot[msk]
        ar = arel[msk]
        fbe = np.zeros((TOT, CDIM), F16)
        fbe[sl] = Fb16[b_idx[msk]]
        selT = np.zeros((P, TOT), F16)
        selT[ar, sl] = 1.0
        sel = np.zeros((P, TOT), F16)
        sel[sl % P, (sl // P) * P + ar] = 1.0

        FaT = np.zeros((CDIM, NPADQ), F16)
        FaT[:, :NAC] = Fa[m * NAC:(m + 1) * NAC].T.astype(F16)
        Fa_res = np.zeros((NPADQ, CDIM), F32)
        Fa_res[:NAC] = Fa[m * NAC:(m + 1) * NAC]
        fbet2 = fbe.T.reshape(2, P, TOT).transpose(1, 0, 2)   # [128, 2, TOT]
        per_core.append({
            "FbET": np.ascontiguousarray(fbet2),
            "SELT": selT,
            "SEL": sel,
            "FaT": FaT,
            "FaRes": Fa_res,
        })

    meta = {"CAP": CAP.astype(int), "coff": coff.astype(int), "TOT": TOT}
    return meta, shared, per_core


def build_program(meta):
    import concourse.bacc as bacc
    import concourse.mybir as mybir
    from concourse.tile import TileContext

    dt = mybir.dt
    nc = bacc.Bacc("TRN2", target_bir_lowering=False, debug=False,
                   num_devices=NCORES)

    CAP, coff, TOT = meta["CAP"], meta["coff"], meta["TOT"]
    CMAX = int(CAP.max()) // P
    AluOp = mybir.AluOpType

    FbET_t = nc.dram_tensor("FbET", [P, 2, TOT], dt.float16, kind="ExternalInput")
    SELT_t = nc.dram_tensor("SELT", [P, TOT], dt.float16, kind="ExternalInput")
    SEL_t = nc.dram_tensor("SEL", [P, TOT], dt.float16, kind="ExternalInput")
    FaT_t = nc.dram_tensor("FaT", [CDIM, NPADQ], dt.float16, kind="ExternalInput")
    FaRes_t = nc.dram_tensor("FaRes", [NPADQ, CDIM], dt.float32, kind="ExternalInput")
    WqT_t = nc.dram_tensor("WqT", [CDIM, CDIM], dt.float16, kind="ExternalInput")
    WKVT_t = nc.dram_tensor("WKVT", [P, 2, 2 * CDIM], dt.float16, kind="ExternalInput")
    WprojT_t = nc.dram_tensor("WprojT", [CDIM, CDIM], dt.float16, kind="ExternalInput")
    IDENT_t = nc.dram_tensor("IDENT", [P, P], dt.float32, kind="ExternalInput")
    OUT_t = nc.dram_tensor("OUT", [NPADQ, CDIM], dt.float32, kind="ExternalOutput")

    with TileContext(nc) as tc:
        with tc.tile_pool(name="res", bufs=1) as rpool:
            wq = rpool.tile([P, 2, CDIM], dt.float16, tag="wq")
            wkv = rpool.tile([P, 2, 2 * CDIM], dt.float16, tag="wkv")
            wproj = rpool.tile([P, 2, CDIM], dt.float16, tag="wproj")
            ident = rpool.tile([P, P], dt.float32, tag="ident")
            nc.sync.dma_start(out=wq[:, 0, :], in_=WqT_t[0:P, :])
            nc.sync.dma_start(out=wq[:, 1, :], in_=WqT_t[P:2 * P, :])
            nc.sync.dma_start(out=wkv[:], in_=WKVT_t[:, :, :])
            nc.sync.dma_start(out=wproj[:, 0, :], in_=WprojT_t[0:P, :])
            nc.sync.dma_start(out=wproj[:, 1, :], in_=WprojT_t[P:2 * P, :])
            nc.sync.dma_start(out=ident[:], in_=IDENT_t[:, :])
            qres = rpool.tile([P, NBLK, CDIM], dt.float16, tag="qres")

            # ---- Phase A: build Q into SBUF ----
            with tc.tile_pool(name="bld", bufs=2) as bpool, \
                 tc.tile_pool(name="psA", bufs=4, space="PSUM") as psA:
                for c0 in range(0, NPADQ, CHUNK):
                    nsub = min(CHUNK, NPADQ - c0) // P
                    ft = bpool.tile([P, 2, CHUNK], dt.float16, tag="ft")
                    nc.sync.dma_start(out=ft[:, 0, :nsub * P],
                                      in_=FaT_t[0:P, c0:c0 + nsub * P])
                    nc.sync.dma_start(out=ft[:, 1, :nsub * P],
                                      in_=FaT_t[P:2 * P, c0:c0 + nsub * P])
                    for s in range(nsub):
                        ps = psA.tile([P, CDIM], dt.float32, tag="psA")
                        nc.tensor.matmul(ps[:], ft[:, 0, s * P:(s + 1) * P],
                                         wq[:, 0, :], start=True, stop=False)
                        nc.tensor.matmul(ps[:], ft[:, 1, s * P:(s + 1) * P],
                                         wq[:, 1, :], start=False, stop=True)
                        nc.scalar.copy(out=qres[:, c0 // P + s, :], in_=ps[:])

            # ---- Phase B: edge attention ----
            with tc.tile_pool(name="gat", bufs=3) as gpool, \
                 tc.tile_pool(name="wrk", bufs=3) as wpool, \
                 tc.tile_pool(name="fin", bufs=2) as fpool, \
                 tc.tile_pool(name="psKV", bufs=2, space="PSUM") as psKV, \
                 tc.tile_pool(name="psQE", bufs=2, space="PSUM") as psQE, \
                 tc.tile_pool(name="psM", bufs=1, space="PSUM") as psM:
                for j in range(NBLK):
                    Cj = int(CAP[j]) // P
                    c0 = int(coff[j])
                    fbet = gpool.tile([P, 2, CMAX * P], dt.float16, tag="fbet")
                    nc.sync.dma_start(out=fbet[:, :, :Cj * P],
                                      in_=FbET_t[:, :, c0:c0 + Cj * P])
                    selt = gpool.tile([P, CMAX * P], dt.float16, tag="selt")
                    nc.sync.dma_start(out=selt[:, :Cj * P],
                                      in_=SELT_t[:, c0:c0 + Cj * P])
                    sel = gpool.tile([P, CMAX * P], dt.float16, tag="sel")
                    nc.sync.dma_start(out=sel[:, :Cj * P],
                                      in_=SEL_t[:, c0:c0 + Cj * P])

                    dn_ps = psM.tile([P, H + CDIM], dt.float32, tag="dn")
                    for g0 in range(0, Cj, G):
                        gn = min(G, Cj - g0)
                        kv_ps = psKV.tile([P, G, 2 * CDIM], dt.float32, tag="kv")
                        qe_ps = psQE.tile([P, G, CDIM], dt.float32, tag="qe")
                        for t in range(g0, g0 + gn):
                            u = t - g0
                            nc.tensor.matmul(kv_ps[:, u, :],
                                             fbet[:, 0, t * P:(t + 1) * P],
                                             wkv[:, 0, :], start=True, stop=False)
                            nc.tensor.matmul(kv_ps[:, u, :],
                                             fbet[:, 1, t * P:(t + 1) * P],
                                             wkv[:, 1, :], start=False, stop=True)
                            nc.tensor.matmul(qe_ps[:, u, :],
                                             selt[:, t * P:(t + 1) * P],
                                             qres[:, j, :], start=True, stop=True)
                        qe_sb = wpool.tile([P, G, CDIM], dt.float16, tag="qe_sb")
                        nc.scalar.copy(out=qe_sb[:, :gn, :], in_=qe_ps[:, :gn, :])
                        k_sb = wpool.tile([P, G, CDIM], dt.float16, tag="k_sb")
                        nc.scalar.copy(out=k_sb[:, :gn, :], in_=kv_ps[:, :gn, 0:CDIM])
                        prod = wpool.tile([P, G, CDIM], dt.float16, tag="prod")
                        nc.gpsimd.tensor_tensor(
                            out=prod[:, :gn, :], in0=qe_sb[:, :gn, :],
                            in1=k_sb[:, :gn, :], op=AluOp.mult)
                        logits = wpool.tile([P, G, H], dt.float32, tag="logits")
                        nc.vector.tensor_reduce(
                            out=logits[:, :gn, :],
                            in_=prod[:, :gn, :].rearrange("p g (h d) -> p g h d", d=DH),
                            axis=mybir.AxisListType.X, op=AluOp.add)
                        exwv = wpool.tile([P, G, H + CDIM], dt.float16, tag="exwv")
                        nc.scalar.activation(
                            out=exwv[:, :gn, 0:H], in_=logits[:, :gn, :],
                            func=mybir.ActivationFunctionType.Exp, scale=SCALE)
                        nc.vector.tensor_tensor(
                            out=exwv[:, :gn, H:H + CDIM],
                            in0=kv_ps[:, :gn, CDIM:2 * CDIM],
                            in1=exwv[:, :gn, 0:H].unsqueeze(3).to_broadcast(
                                [P, gn, H, DH]),
                            op=AluOp.mult)
                        for t in range(g0, g0 + gn):
                            u = t - g0
                            nc.tensor.matmul(dn_ps[:], sel[:, t * P:(t + 1) * P],
                                             exwv[:, u, :],
                                             start=(t == 0), stop=(t == Cj - 1))

                    # block finalize
                    fin = psM.tile([P, 2 * CDIM], dt.float32, tag="fin")
                    den = fpool.tile([P, H], dt.float32, tag="den")
                    nc.vector.tensor_scalar_max(out=den[:], in0=dn_ps[:, 0:H],
                                                scalar1=1e-30)
                    rec = fpool.tile([P, H], dt.float32, tag="rec")
                    nc.vector.reciprocal(out=rec[:], in_=den[:])
                    s_sb = fpool.tile([P, CDIM], dt.float32, tag="s_sb")
                    nc.vector.tensor_tensor(
                        out=s_sb[:], in0=dn_ps[:, H:H + CDIM],
                        in1=rec[:].unsqueeze(2).to_broadcast([P, H, DH]),
                        op=AluOp.mult)
                    nc.tensor.transpose(fin[:, 0:P], s_sb[:, 0:P], ident[:])
                    nc.tensor.transpose(fin[:, P:2 * P], s_sb[:, P:2 * P], ident[:])
                    st_sb = fpool.tile([P, 2, P], dt.float16, tag="st_sb")
                    nc.scalar.copy(out=st_sb[:], in_=fin[:, 0:2 * P])
                    nc.tensor.matmul(fin[:, CDIM:2 * CDIM], st_sb[:, 0, :],
                                     wproj[:, 0, :], start=True, stop=False)
                    nc.tensor.matmul(fin[:, CDIM:2 * CDIM], st_sb[:, 1, :],
                                     wproj[:, 1, :], start=False, stop=True)
                    fa_t = fpool.tile([P, CDIM], dt.float32, tag="fa_t")
                    nc.sync.dma_start(out=fa_t[:], in_=FaRes_t[j * P:(j + 1) * P, :])
                    res = fpool.tile([P, CDIM], dt.float32, tag="res")
                    nc.vector.tensor_tensor(out=res[:], in0=fin[:, CDIM:2 * CDIM],
                                            in1=fa_t[:], op=AluOp.add)
                    nc.sync.dma_start(out=OUT_t[j * P:(j + 1) * P, :], in_=res[:])

    nc.compile()
    return nc


TRACE = False          # set by test harness for NTFF profiling
LAST_RESULT = None     # BassKernelResults of the last run (for profiling)


def kernel(**inputs):
    global LAST_RESULT
    from concourse.bass_utils import run_bass_kernel_spmd

    meta, shared, per_core = preprocess(**inputs)
    nc = build_program(meta)
    in_maps = [dict(shared, **pc) for pc in per_core]
    res = run_bass_kernel_spmd(nc, in_maps, core_ids=list(range(NCORES)),
                               trace=TRACE)
    LAST_RESULT = res
    out = np.empty((NA, CDIM), F32)
    for m in range(NCORES):
        out[m * NAC:(m + 1) * NAC] = res.results[m]["OUT"][:NAC]
    return out


# revision 11
# speedup vs baseline: 1.3441x; 1.1695x over previous
"""Trainium2 Bass kernel for sparse multi-head edge attention (V3).

Computation (per the nn.Module):
    Q = Fa @ Wq.T, K = Fb @ Wk.T, V = Fb @ Wv.T   (H=8 heads x 32)
    per edge e: logit[e,h] = <Q[a_e,h,:], K[b_e,h,:]> / sqrt(32)
    segmented softmax over edges per query, out = Fa + (softmax(V)) @ Wproj.T

Strategy (8 NeuronCores, SPMD, query-sharded, no collectives):
  - Core m owns queries [m*6250, (m+1)*6250).  Edges are routed to the owner
    of their query, grouped by 128-query block, padded to 128-edge tiles.
  - NO gather anywhere: the host distributes per-edge Fb rows (edge-sharded
    input layout, FbET = Fb[b_e].T in slot order) and the kernel recomputes
    K|V per edge with one fused matmul pair per 128-edge tile
    (KVe = FbE @ [Wk.T|Wv.T]).  This trades 2x matmul FLOPs for zero GPSIMD
    descriptor generation and no DRAM KV table round-trip — the previous
    dma_gather design was bottlenecked by ~10ns/row descriptor costs.
  - Per-edge Q rows come from a one-hot selection matmul (selT.T @ Qblk)
    with host-precomputed one-hot SEL/SELT streams (also encodes padding:
    pad slots have all-zero one-hot columns so they contribute exactly 0).
  - Work is spread across all four compute engines and batched in groups of
    G=2 tiles so per-instruction overheads amortize:
      PE:     KV matmuls, Qe selection, [den|num] accumulation, projection
      Scalar: Qe and K PSUM->SBUF copies, exp()
      DVE:    Q*K product, per-head logit reduction, exp-weighted V
  - Softmax without max-subtraction (|logit| <~ 10 for this operator family,
    fp32/fp16 exp is safe); den clamped like the reference.
"""

import math

import numpy as np

P = 128
H = 8
DH = 32
CDIM = 256
NA = 50000
NB = 50000
NCORES = 8
NAC = NA // NCORES          # 6250 queries per core
NBLK = (NAC + P - 1) // P   # 49 query blocks per core
NPADQ = NBLK * P            # 6272 padded queries per core
CHUNK = 2048
G = 2                       # tiles per op-batching group
SCALE = 1.0 / math.sqrt(DH)

F16 = np.float16
F32 = np.float32


def _ceil128(x):
    return (np.asarray(x) + P - 1) // P * P


def preprocess(Fa, Fb, a_idx, b_idx, Wq, Wk, Wv, Wproj):
    """Host-side sharding: returns (meta, shared_inputs, per_core_inputs)."""
    a_idx = np.asarray(a_idx).astype(np.int64)
    b_idx = np.asarray(b_idx).astype(np.int64)
    Fa = np.asarray(Fa, F32)
    Fb = np.asarray(Fb, F32)

    core = a_idx // NAC
    a_loc = a_idx - core * NAC
    blk = a_loc // P
    arel = a_loc % P

    cnt = np.bincount(core * NBLK + blk, minlength=NCORES * NBLK)
    cnt = cnt.reshape(NCORES, NBLK)
    CAP = np.maximum(_ceil128(cnt.max(axis=0)), P)
    coff = np.concatenate([[0], np.cumsum(CAP)])
    TOT = int(coff[-1])

    # rank of each edge within its (core, blk) group
    ne = a_idx.shape[0]
    gid = core * NBLK + blk
    order = np.argsort(gid, kind="stable")
    counts = np.bincount(gid, minlength=NCORES * NBLK)
    gstart = np.concatenate([[0], np.cumsum(counts)])[:-1]
    rank = np.empty(ne, np.int64)
    rank[order] = np.arange(ne) - gstart[gid[order]]
    slot = coff[blk] + rank

    Fb16 = Fb.astype(F16)
    wkvT = np.concatenate([Wk.T, Wv.T], axis=1)          # [256, 512]
    shared = {
        "WqT": Wq.T.astype(F16).copy(),
        # contraction-half layout [k=128, half=2, n=512]
        "WKVT": wkvT.reshape(2, P, 2 * CDIM).transpose(1, 0, 2).astype(F16).copy(),
        "WprojT": Wproj.T.astype(F16).copy(),
        "IDENT": np.eye(P, dtype=F32),
    }

    per_core = []
    for m in range(NCORES):
        msk = core == m
        sl = slot[msk]
        ar = arel[msk]
        fbe = np.zeros((TOT, CDIM), F16)
        fbe[sl] = Fb16[b_idx[msk]]
        selT = np.zeros((P, TOT), F16)
        selT[ar, sl] = 1.0
        sel = np.zeros((P, TOT), F16)
        sel[sl % P, (sl // P) * P + ar] = 1.0

        FaT = np.zeros((CDIM, NPADQ), F16)
        FaT[:, :NAC] = Fa[m * NAC:(m + 1) * NAC].T.astype(F16)
        Fa_res = np.zeros((NPADQ, CDIM), F32)
        Fa_res[:NAC] = Fa[m * NAC:(m + 1) * NAC]
        fbet2 = fbe.T.reshape(2, P, TOT).transpose(1, 0, 2)   # [128, 2, TOT]
        per_core.append({
            "FbET": np.ascontiguousarray(fbet2),
            "SELT": selT,
            "SEL": sel,
            "FaT": FaT,
            "FaRes": Fa_res,
        })

    meta = {"CAP": CAP.astype(int), "coff": coff.astype(int), "TOT": TOT}
    return meta, shared, per_core


def build_program(meta):
    import concourse.bacc as bacc
    import concourse.mybir as mybir
    from concourse.tile import TileContext

    dt = mybir.dt
    nc = bacc.Bacc("TRN2", target_bir_lowering=False, debug=False,
                   num_devices=NCORES)

    CAP, coff, TOT = meta["CAP"], meta["coff"], meta["TOT"]
    CMAX = int(CAP.max()) // P
    AluOp = mybir.AluOpType

    FbET_t = nc.dram_tensor("FbET", [P, 2, TOT], dt.float16, kind="ExternalInput")
    SELT_t = nc.dram_tensor("SELT", [P, TOT], dt.float16, kind="ExternalInput")
    SEL_t = nc.dram_tensor("SEL", [P, TOT], dt.float16, kind="ExternalInput")
    FaT_t = nc.dram_tensor("FaT", [CDIM, NPADQ], dt.float16, kind="ExternalInput")
    FaRes_t = nc.dram_tensor("FaRes", [NPADQ, CDIM], dt.float32, kind="ExternalInput")
    WqT_t = nc.dram_tensor("WqT", [CDIM, CDIM], dt.float16, kind="ExternalInput")
    WKVT_t = nc.dram_tensor("WKVT", [P, 2, 2 * CDIM], dt.float16, kind="ExternalInput")
    WprojT_t = nc.dram_tensor("WprojT", [CDIM, CDIM], dt.float16, kind="ExternalInput")
    IDENT_t = nc.dram_tensor("IDENT", [P, P], dt.float32, kind="ExternalInput")
    OUT_t = nc.dram_tensor("OUT", [NPADQ, CDIM], dt.float32, kind="ExternalOutput")

    with TileContext(nc) as tc:
        with tc.tile_pool(name="res", bufs=1) as rpool:
            wq = rpool.tile([P, 2, CDIM], dt.float16, tag="wq")
            wkv = rpool.tile([P, 2, 2 * CDIM], dt.float16, tag="wkv")
            wproj = rpool.tile([P, 2, CDIM], dt.float16, tag="wproj")
            ident = rpool.tile([P, P], dt.float32, tag="ident")
            nc.sync.dma_start(out=wq[:, 0, :], in_=WqT_t[0:P, :])
            nc.sync.dma_start(out=wq[:, 1, :], in_=WqT_t[P:2 * P, :])
            nc.sync.dma_start(out=wkv[:], in_=WKVT_t[:, :, :])
            nc.sync.dma_start(out=wproj[:, 0, :], in_=WprojT_t[0:P, :])
            nc.sync.dma_start(out=wproj[:, 1, :], in_=WprojT_t[P:2 * P, :])
            nc.sync.dma_start(out=ident[:], in_=IDENT_t[:, :])
            qres = rpool.tile([P, NBLK, CDIM], dt.float16, tag="qres")

            # ---- Phase A: build Q into SBUF ----
            with tc.tile_pool(name="bld", bufs=2) as bpool, \
                 tc.tile_pool(name="psA", bufs=4, space="PSUM") as psA:
                for c0 in range(0, NPADQ, CHUNK):
                    nsub = min(CHUNK, NPADQ - c0) // P
                    ft = bpool.tile([P, 2, CHUNK], dt.float16, tag="ft")
                    nc.sync.dma_start(out=ft[:, 0, :nsub * P],
                                      in_=FaT_t[0:P, c0:c0 + nsub * P])
                    nc.sync.dma_start(out=ft[:, 1, :nsub * P],
                                      in_=FaT_t[P:2 * P, c0:c0 + nsub * P])
                    for s in range(nsub):
                        ps = psA.tile([P, CDIM], dt.float32, tag="psA")
                        nc.tensor.matmul(ps[:], ft[:, 0, s * P:(s + 1) * P],
                                         wq[:, 0, :], start=True, stop=False)
                        nc.tensor.matmul(ps[:], ft[:, 1, s * P:(s + 1) * P],
                                         wq[:, 1, :], start=False, stop=True)
                        nc.scalar.copy(out=qres[:, c0 // P + s, :], in_=ps[:])

            # ---- Phase B: edge attention ----
            with tc.tile_pool(name="gat", bufs=3) as gpool, \
                 tc.tile_pool(name="wrk", bufs=3) as wpool, \
                 tc.tile_pool(name="fin", bufs=2) as fpool, \
                 tc.tile_pool(name="psKV", bufs=2, space="PSUM") as psKV, \
                 tc.tile_pool(name="psQE", bufs=2, space="PSUM") as psQE, \
                 tc.tile_pool(name="psM", bufs=1, space="PSUM") as psM:
                for j in range(NBLK):
                    Cj = int(CAP[j]) // P
                    c0 = int(coff[j])
                    fbet = gpool.tile([P, 2, CMAX * P], dt.float16, tag="fbet")
                    nc.sync.dma_start(out=fbet[:, :, :Cj * P],
                                      in_=FbET_t[:, :, c0:c0 + Cj * P])
                    selt = gpool.tile([P, CMAX * P], dt.float16, tag="selt")
                    nc.sync.dma_start(out=selt[:, :Cj * P],
                                      in_=SELT_t[:, c0:c0 + Cj * P])
                    sel = gpool.tile([P, CMAX * P], dt.float16, tag="sel")
                    nc.sync.dma_start(out=sel[:, :Cj * P],
                                      in_=SEL_t[:, c0:c0 + Cj * P])

                    dn_ps = psM.tile([P, H + CDIM], dt.float32, tag="dn")
                    for g0 in range(0, Cj, G):
                        gn = min(G, Cj - g0)
                        kv_ps = psKV.tile([P, G, 2 * CDIM], dt.float32, tag="kv")
                        qe_ps = psQE.tile([P, G, CDIM], dt.float32, tag="qe")
                        for t in range(g0, g0 + gn):
                            u = t - g0
                            nc.tensor.matmul(kv_ps[:, u, :],
                                             fbet[:, 0, t * P:(t + 1) * P],
                                             wkv[:, 0, :], start=True, stop=False)
                            nc.tensor.matmul(kv_ps[:, u, :],
                                             fbet[:, 1, t * P:(t + 1) * P],
                                             wkv[:, 1, :], start=False, stop=True)
                            nc.tensor.matmul(qe_ps[:, u, :],
                                             selt[:, t * P:(t + 1) * P],
                                             qres[:, j, :], start=True, stop=True)
                        qe_sb = wpool.tile([P, G, CDIM], dt.float16, tag="qe_sb")
                        nc.scalar.copy(out=qe_sb[:, :gn, :], in_=qe_ps[:, :gn, :])
                        k_sb = wpool.tile([P, G, CDIM], dt.float16, tag="k_sb")
                        nc.scalar.copy(out=k_sb[:, :gn, :], in_=kv_ps[:, :gn, 0:CDIM])
                        prod = wpool.tile([P, G, CDIM], dt.float16, tag="prod")
                        nc.vector.tensor_tensor(
                            out=prod[:, :gn, :], in0=qe_sb[:, :gn, :],
                            in1=k_sb[:, :gn, :], op=AluOp.mult)
                        logits = wpool.tile([P, G, H], dt.float32, tag="logits")
                        nc.vector.tensor_reduce(
                            out=logits[:, :gn, :],
                            in_=prod[:, :gn, :].rearrange("p g (h d) -> p g h d", d=DH),
                            axis=mybir.AxisListType.X, op=AluOp.add)
                        exwv = wpool.tile([P, G, H + CDIM], dt.float16, tag="exwv")
                        nc.scalar.activation(
                            out=exwv[:, :gn, 0:H], in_=logits[:, :gn, :],
                            func=mybir.ActivationFunctionType.Exp, scale=SCALE)
                        nc.vector.tensor_tensor(
                            out=exwv[:, :gn, H:H + CDIM],
                            in0=kv_ps[:, :gn, CDIM:2 * CDIM],
                            in1=exwv[:, :gn, 0:H].unsqueeze(3).to_broadcast(
                                [P, gn, H, DH]),
                            op=AluOp.mult)
                        for t in range(g0, g0 + gn):
                            u = t - g0
                            nc.tensor.matmul(dn_ps[:], sel[:, t * P:(t + 1) * P],
                                             exwv[:, u, :],
                                             start=(t == 0), stop=(t == Cj - 1))

                    # block finalize
                    fin = psM.tile([P, 2 * CDIM], dt.float32, tag="fin")
                    den = fpool.tile([P, H], dt.float32, tag="den")
                    nc.vector.tensor_scalar_max(out=den[:], in0=dn_ps[:, 0:H],
                                                scalar1=1e-30)
                    rec = fpool.tile([P, H], dt.float32, tag="rec")
                    nc.vector.reciprocal(out=rec[:], in_=den[:])
                    s_sb = fpool.tile([P, CDIM], dt.float32, tag="s_sb")
                    nc.vector.tensor_tensor(
                        out=s_sb[:], in0=dn_ps[:, H:H + CDIM],
                        in1=rec[:].unsqueeze(2).to_broadcast([P, H, DH]),
                        op=AluOp.mult)
                    nc.tensor.transpose(fin[:, 0:P], s_sb[:, 0:P], ident[:])
                    nc.tensor.transpose(fin[:, P:2 * P], s_sb[:, P:2 * P], ident[:])
                    st_sb = fpool.tile([P, 2, P], dt.float16, tag="st_sb")
                    nc.scalar.copy(out=st_sb[:], in_=fin[:, 0:2 * P])
                    nc.tensor.matmul(fin[:, CDIM:2 * CDIM], st_sb[:, 0, :],
                                     wproj[:, 0, :], start=True, stop=False)
                    nc.tensor.matmul(fin[:, CDIM:2 * CDIM], st_sb[:, 1, :],
                                     wproj[:, 1, :], start=False, stop=True)
                    fa_t = fpool.tile([P, CDIM], dt.float32, tag="fa_t")
                    nc.sync.dma_start(out=fa_t[:], in_=FaRes_t[j * P:(j + 1) * P, :])
                    res = fpool.tile([P, CDIM], dt.float32, tag="res")
                    nc.vector.tensor_tensor(out=res[:], in0=fin[:, CDIM:2 * CDIM],
                                            in1=fa_t[:], op=AluOp.add)
                    nc.sync.dma_start(out=OUT_t[j * P:(j + 1) * P, :], in_=res[:])

    nc.compile()
    return nc


TRACE = False          # set by test harness for NTFF profiling
LAST_RESULT = None     # BassKernelResults of the last run (for profiling)


def kernel(**inputs):
    global LAST_RESULT
    from concourse.bass_utils import run_bass_kernel_spmd

    meta, shared, per_core = preprocess(**inputs)
    nc = build_program(meta)
    in_maps = [dict(shared, **pc) for pc in per_core]
    res = run_bass_kernel_spmd(nc, in_maps, core_ids=list(range(NCORES)),
                               trace=TRACE)
    LAST_RESULT = res
    out = np.empty((NA, CDIM), F32)
    for m in range(NCORES):
        out[m * NAC:(m + 1) * NAC] = res.results[m]["OUT"][:NAC]
    return out


# revision 12
# speedup vs baseline: 1.6227x; 1.2073x over previous
"""Trainium2 Bass kernel for sparse multi-head edge attention (V3).

Computation (per the nn.Module):
    Q = Fa @ Wq.T, K = Fb @ Wk.T, V = Fb @ Wv.T   (H=8 heads x 32)
    per edge e: logit[e,h] = <Q[a_e,h,:], K[b_e,h,:]> / sqrt(32)
    segmented softmax over edges per query, out = Fa + (softmax(V)) @ Wproj.T

Strategy (8 NeuronCores, SPMD, query-sharded, no collectives):
  - Core m owns queries [m*6250, (m+1)*6250).  Edges are routed to the owner
    of their query, grouped by 128-query block, padded to 128-edge tiles.
  - NO gather anywhere: the host distributes per-edge Fb rows (edge-sharded
    input layout, FbET = Fb[b_e].T in slot order) and the kernel recomputes
    K|V per edge with one fused matmul pair per 128-edge tile
    (KVe = FbE @ [Wk.T|Wv.T]).  This trades 2x matmul FLOPs for zero GPSIMD
    descriptor generation and no DRAM KV table round-trip — the previous
    dma_gather design was bottlenecked by ~10ns/row descriptor costs.
  - Per-edge Q rows come from a one-hot selection matmul (selT.T @ Qblk)
    with host-precomputed one-hot SEL/SELT streams (also encodes padding:
    pad slots have all-zero one-hot columns so they contribute exactly 0).
  - Work is spread across all four compute engines and batched in groups of
    G=2 tiles so per-instruction overheads amortize:
      PE:     KV matmuls, Qe selection, [den|num] accumulation, projection
      Scalar: Qe and K PSUM->SBUF copies, exp()
      DVE:    Q*K product, per-head logit reduction, exp-weighted V
  - Softmax without max-subtraction (|logit| <~ 10 for this operator family,
    fp32/fp16 exp is safe); den clamped like the reference.
"""

import math

import numpy as np

P = 128
H = 8
DH = 32
CDIM = 256
NA = 50000
NB = 50000
NCORES = 8
NAC = NA // NCORES          # 6250 queries per core
NBLK = (NAC + P - 1) // P   # 49 query blocks per core
NPADQ = NBLK * P            # 6272 padded queries per core
CHUNK = 2048
G = 2                       # tiles per op-batching group
SCALE = 1.0 / math.sqrt(DH)

F16 = np.float16
F32 = np.float32


def _ceil128(x):
    return (np.asarray(x) + P - 1) // P * P


def preprocess(Fa, Fb, a_idx, b_idx, Wq, Wk, Wv, Wproj):
    """Host-side sharding: returns (meta, shared_inputs, per_core_inputs)."""
    a_idx = np.asarray(a_idx).astype(np.int64)
    b_idx = np.asarray(b_idx).astype(np.int64)
    Fa = np.asarray(Fa, F32)
    Fb = np.asarray(Fb, F32)

    core = a_idx // NAC
    a_loc = a_idx - core * NAC
    blk = a_loc // P
    arel = a_loc % P

    cnt = np.bincount(core * NBLK + blk, minlength=NCORES * NBLK)
    cnt = cnt.reshape(NCORES, NBLK)
    CAP = np.maximum(_ceil128(cnt.max(axis=0)), P)
    coff = np.concatenate([[0], np.cumsum(CAP)])
    TOT = int(coff[-1])

    # rank of each edge within its (core, blk) group
    ne = a_idx.shape[0]
    gid = core * NBLK + blk
    order = np.argsort(gid, kind="stable")
    counts = np.bincount(gid, minlength=NCORES * NBLK)
    gstart = np.concatenate([[0], np.cumsum(counts)])[:-1]
    rank = np.empty(ne, np.int64)
    rank[order] = np.arange(ne) - gstart[gid[order]]
    slot = coff[blk] + rank

    Fb16 = Fb.astype(F16)
    wkvT = np.concatenate([Wk.T, Wv.T], axis=1)          # [256, 512]
    shared = {
        "WqT": Wq.T.astype(F16).copy(),
        # contraction-half layout [k=128, half=2, n=512]
        "WKVT": wkvT.reshape(2, P, 2 * CDIM).transpose(1, 0, 2).astype(F16).copy(),
        "WprojT": Wproj.T.astype(F16).copy(),
        "IDENT": np.eye(P, dtype=F32),
    }

    per_core = []
    for m in range(NCORES):
        msk = core == m
        sl = slot[msk]
        ar = arel[msk]
        fbe = np.zeros((TOT, CDIM), F16)
        fbe[sl] = Fb16[b_idx[msk]]
        selT = np.zeros((P, TOT), F16)
        selT[ar, sl] = 1.0
        sel = np.zeros((P, TOT), F16)
        sel[sl % P, (sl // P) * P + ar] = 1.0

        FaT = np.zeros((CDIM, NPADQ), F16)
        FaT[:, :NAC] = Fa[m * NAC:(m + 1) * NAC].T.astype(F16)
        Fa_res = np.zeros((NPADQ, CDIM), F32)
        Fa_res[:NAC] = Fa[m * NAC:(m + 1) * NAC]
        fbet2 = fbe.T.reshape(2, P, TOT).transpose(1, 0, 2)   # [128, 2, TOT]
        per_core.append({
            "FbET": np.ascontiguousarray(fbet2),
            "SELT": selT,
            "SEL": sel,
            "FaT": FaT,
            "FaRes": Fa_res,
        })

    meta = {"CAP": CAP.astype(int), "coff": coff.astype(int), "TOT": TOT}
    return meta, shared, per_core


def build_program(meta):
    import concourse.bacc as bacc
    import concourse.mybir as mybir
    from concourse.tile import TileContext

    dt = mybir.dt
    nc = bacc.Bacc("TRN2", target_bir_lowering=False, debug=False,
                   num_devices=NCORES)

    CAP, coff, TOT = meta["CAP"], meta["coff"], meta["TOT"]
    CMAX = int(CAP.max()) // P
    AluOp = mybir.AluOpType

    FbET_t = nc.dram_tensor("FbET", [P, 2, TOT], dt.float16, kind="ExternalInput")
    SELT_t = nc.dram_tensor("SELT", [P, TOT], dt.float16, kind="ExternalInput")
    SEL_t = nc.dram_tensor("SEL", [P, TOT], dt.float16, kind="ExternalInput")
    FaT_t = nc.dram_tensor("FaT", [CDIM, NPADQ], dt.float16, kind="ExternalInput")
    FaRes_t = nc.dram_tensor("FaRes", [NPADQ, CDIM], dt.float32, kind="ExternalInput")
    WqT_t = nc.dram_tensor("WqT", [CDIM, CDIM], dt.float16, kind="ExternalInput")
    WKVT_t = nc.dram_tensor("WKVT", [P, 2, 2 * CDIM], dt.float16, kind="ExternalInput")
    WprojT_t = nc.dram_tensor("WprojT", [CDIM, CDIM], dt.float16, kind="ExternalInput")
    IDENT_t = nc.dram_tensor("IDENT", [P, P], dt.float32, kind="ExternalInput")
    OUT_t = nc.dram_tensor("OUT", [NPADQ, CDIM], dt.float32, kind="ExternalOutput")

    with TileContext(nc) as tc:
        with tc.tile_pool(name="res", bufs=1) as rpool:
            wq = rpool.tile([P, 2, CDIM], dt.float16, tag="wq")
            wkv = rpool.tile([P, 2, 2 * CDIM], dt.float16, tag="wkv")
            wproj = rpool.tile([P, 2, CDIM], dt.float16, tag="wproj")
            ident = rpool.tile([P, P], dt.float32, tag="ident")
            nc.sync.dma_start(out=wq[:, 0, :], in_=WqT_t[0:P, :])
            nc.sync.dma_start(out=wq[:, 1, :], in_=WqT_t[P:2 * P, :])
            nc.sync.dma_start(out=wkv[:], in_=WKVT_t[:, :, :])
            nc.sync.dma_start(out=wproj[:, 0, :], in_=WprojT_t[0:P, :])
            nc.sync.dma_start(out=wproj[:, 1, :], in_=WprojT_t[P:2 * P, :])
            nc.sync.dma_start(out=ident[:], in_=IDENT_t[:, :])
            qres = rpool.tile([P, NBLK, CDIM], dt.float16, tag="qres")

            # ---- Phase A: build Q into SBUF ----
            with tc.tile_pool(name="bld", bufs=2) as bpool, \
                 tc.tile_pool(name="psA", bufs=4, space="PSUM") as psA:
                for c0 in range(0, NPADQ, CHUNK):
                    nsub = min(CHUNK, NPADQ - c0) // P
                    ft = bpool.tile([P, 2, CHUNK], dt.float16, tag="ft")
                    nc.sync.dma_start(out=ft[:, 0, :nsub * P],
                                      in_=FaT_t[0:P, c0:c0 + nsub * P])
                    nc.sync.dma_start(out=ft[:, 1, :nsub * P],
                                      in_=FaT_t[P:2 * P, c0:c0 + nsub * P])
                    for s in range(nsub):
                        ps = psA.tile([P, CDIM], dt.float32, tag="psA")
                        nc.tensor.matmul(ps[:], ft[:, 0, s * P:(s + 1) * P],
                                         wq[:, 0, :], start=True, stop=False)
                        nc.tensor.matmul(ps[:], ft[:, 1, s * P:(s + 1) * P],
                                         wq[:, 1, :], start=False, stop=True)
                        nc.scalar.copy(out=qres[:, c0 // P + s, :], in_=ps[:])

            # ---- Phase B: edge attention ----
            with tc.tile_pool(name="gat", bufs=4) as gpool, \
                 tc.tile_pool(name="wrk", bufs=6) as wpool, \
                 tc.tile_pool(name="fin", bufs=3) as fpool, \
                 tc.tile_pool(name="psKV", bufs=2, space="PSUM") as psKV, \
                 tc.tile_pool(name="psQE", bufs=1, space="PSUM") as psQE, \
                 tc.tile_pool(name="psDN", bufs=2, space="PSUM") as psDN, \
                 tc.tile_pool(name="psFIN", bufs=1, space="PSUM") as psFIN:
                for j in range(NBLK):
                    Cj = int(CAP[j]) // P
                    c0 = int(coff[j])
                    fbet = gpool.tile([P, 2, CMAX * P], dt.float16, tag="fbet")
                    nc.sync.dma_start(out=fbet[:, :, :Cj * P],
                                      in_=FbET_t[:, :, c0:c0 + Cj * P])
                    selt = gpool.tile([P, CMAX * P], dt.float16, tag="selt")
                    nc.sync.dma_start(out=selt[:, :Cj * P],
                                      in_=SELT_t[:, c0:c0 + Cj * P])
                    sel = gpool.tile([P, CMAX * P], dt.float16, tag="sel")
                    nc.sync.dma_start(out=sel[:, :Cj * P],
                                      in_=SEL_t[:, c0:c0 + Cj * P])

                    dn_ps = psDN.tile([P, H + CDIM], dt.float32, tag="dn")
                    for g0 in range(0, Cj, G):
                        gn = min(G, Cj - g0)
                        kv_ps = psKV.tile([P, G, 2 * CDIM], dt.float32, tag="kv")
                        qe_ps = psQE.tile([P, G, CDIM], dt.float32, tag="qe")
                        for t in range(g0, g0 + gn):
                            u = t - g0
                            nc.tensor.matmul(kv_ps[:, u, :],
                                             fbet[:, 0, t * P:(t + 1) * P],
                                             wkv[:, 0, :], start=True, stop=False)
                            nc.tensor.matmul(kv_ps[:, u, :],
                                             fbet[:, 1, t * P:(t + 1) * P],
                                             wkv[:, 1, :], start=False, stop=True)
                            nc.tensor.matmul(qe_ps[:, u, :],
                                             selt[:, t * P:(t + 1) * P],
                                             qres[:, j, :], start=True, stop=True)
                        qe_sb = wpool.tile([P, G, CDIM], dt.float16, tag="qe_sb")
                        nc.scalar.copy(out=qe_sb[:, :gn, :], in_=qe_ps[:, :gn, :])
                        k_sb = wpool.tile([P, G, CDIM], dt.float16, tag="k_sb")
                        nc.scalar.copy(out=k_sb[:, :gn, :], in_=kv_ps[:, :gn, 0:CDIM])
                        prod = wpool.tile([P, G, CDIM], dt.float16, tag="prod")
                        nc.vector.tensor_tensor(
                            out=prod[:, :gn, :], in0=qe_sb[:, :gn, :],
                            in1=k_sb[:, :gn, :], op=AluOp.mult)
                        logits = wpool.tile([P, G, H], dt.float32, tag="logits")
                        nc.vector.tensor_reduce(
                            out=logits[:, :gn, :],
                            in_=prod[:, :gn, :].rearrange("p g (h d) -> p g h d", d=DH),
                            axis=mybir.AxisListType.X, op=AluOp.add)
                        exwv = wpool.tile([P, G, H + CDIM], dt.float16, tag="exwv")
                        nc.scalar.activation(
                            out=exwv[:, :gn, 0:H], in_=logits[:, :gn, :],
                            func=mybir.ActivationFunctionType.Exp, scale=SCALE)
                        nc.vector.tensor_tensor(
                            out=exwv[:, :gn, H:H + CDIM],
                            in0=kv_ps[:, :gn, CDIM:2 * CDIM],
                            in1=exwv[:, :gn, 0:H].unsqueeze(3).to_broadcast(
                                [P, gn, H, DH]),
                            op=AluOp.mult)
                        for t in range(g0, g0 + gn):
                            u = t - g0
                            nc.tensor.matmul(dn_ps[:], sel[:, t * P:(t + 1) * P],
                                             exwv[:, u, :],
                                             start=(t == 0), stop=(t == Cj - 1))

                    # block finalize
                    fin = psFIN.tile([P, 2 * CDIM], dt.float32, tag="fin")
                    den = fpool.tile([P, H], dt.float32, tag="den")
                    nc.vector.tensor_scalar_max(out=den[:], in0=dn_ps[:, 0:H],
                                                scalar1=1e-30)
                    rec = fpool.tile([P, H], dt.float32, tag="rec")
                    nc.vector.reciprocal(out=rec[:], in_=den[:])
                    s_sb = fpool.tile([P, CDIM], dt.float32, tag="s_sb")
                    nc.vector.tensor_tensor(
                        out=s_sb[:], in0=dn_ps[:, H:H + CDIM],
                        in1=rec[:].unsqueeze(2).to_broadcast([P, H, DH]),
                        op=AluOp.mult)
                    nc.tensor.transpose(fin[:, 0:P], s_sb[:, 0:P], ident[:])
                    nc.tensor.transpose(fin[:, P:2 * P], s_sb[:, P:2 * P], ident[:])
                    st_sb = fpool.tile([P, 2, P], dt.float16, tag="st_sb")
                    nc.scalar.copy(out=st_sb[:], in_=fin[:, 0:2 * P])
                    nc.tensor.matmul(fin[:, CDIM:2 * CDIM], st_sb[:, 0, :],
                                     wproj[:, 0, :], start=True, stop=False)
                    nc.tensor.matmul(fin[:, CDIM:2 * CDIM], st_sb[:, 1, :],
                                     wproj[:, 1, :], start=False, stop=True)
                    fa_t = fpool.tile([P, CDIM], dt.float32, tag="fa_t")
                    nc.sync.dma_start(out=fa_t[:], in_=FaRes_t[j * P:(j + 1) * P, :])
                    res = fpool.tile([P, CDIM], dt.float32, tag="res")
                    nc.vector.tensor_tensor(out=res[:], in0=fin[:, CDIM:2 * CDIM],
                                            in1=fa_t[:], op=AluOp.add)
                    nc.sync.dma_start(out=OUT_t[j * P:(j + 1) * P, :], in_=res[:])

    nc.compile()
    return nc


TRACE = False          # set by test harness for NTFF profiling
LAST_RESULT = None     # BassKernelResults of the last run (for profiling)


def kernel(**inputs):
    global LAST_RESULT
    from concourse.bass_utils import run_bass_kernel_spmd

    meta, shared, per_core = preprocess(**inputs)
    nc = build_program(meta)
    in_maps = [dict(shared, **pc) for pc in per_core]
    res = run_bass_kernel_spmd(nc, in_maps, core_ids=list(range(NCORES)),
                               trace=TRACE)
    LAST_RESULT = res
    out = np.empty((NA, CDIM), F32)
    for m in range(NCORES):
        out[m * NAC:(m + 1) * NAC] = res.results[m]["OUT"][:NAC]
    return out
